# revision 7
# baseline (speedup 1.0000x reference)
"""Bass/Trainium2 kernel v4: staircase units + PE tiling + paired-unit relu1.

vs kernel3: units of the same band that are j-adjacent are processed as
PAIR-items whose relu1 ops run at FD=896 covering both units in one DVE op
(~1.5x the cost of one 448 op instead of 2x), cutting the DVE relu1 wall.
Leftover pairs are converted to singles so every core runs an identical
(PP pair-items, SS single-items) program. Scores of a pair's two units share
one PSUM bank via a +16 lhsT row offset for the right unit.
"""

import numpy as np
import ml_dtypes

D = 64
IB = 64
JW = 448
N_CORES = 8
BF16 = ml_dtypes.bfloat16

RELU2_ACT = {0, 1, 2, 3, 4, 5, 6, 7}  # relu2 op k on ACT iff k%8 in set
EVAC_ACT = set()

_CACHE = {}


def _build_bass(PP, SS):
    import concourse.bacc as bacc
    import concourse.mybir as mybir
    from concourse.tile import TileContext

    bf16 = mybir.dt.bfloat16
    f32 = mybir.dt.float32

    nc = bacc.Bacc(target_bir_lowering=False)

    add = mybir.AluOpType.add
    vmax = mybir.AluOpType.max
    Relu = mybir.ActivationFunctionType.Relu
    Copy = mybir.ActivationFunctionType.Copy

    NS = IB // 8   # 8 mm1 spans per unit
    NT = IB // 16  # 4 mm2 spans per unit
    NI = PP + SS
    hjw = PP * 2 * JW + SS * JW

    hj_pack = nc.dram_tensor("hj_pack", [128, hjw], bf16, kind="ExternalInput")
    cw = nc.dram_tensor("cw", [128, 32 + 256], bf16, kind="ExternalInput")
    cpk = nc.dram_tensor("cpk", [128, NI * 32 + 1], f32, kind="ExternalInput")
    out = nc.dram_tensor("out", [NI * 128, JW], bf16, kind="ExternalOutput")

    def hj_off(it):  # col offset of item it in hj_pack
        return min(it, PP) * 2 * JW + max(it - PP, 0) * JW

    with TileContext(nc) as tc:
        with (
            tc.tile_pool(name="const", bufs=1) as cpool,
            tc.tile_pool(name="r1p", bufs=14) as r1ppool,
            tc.tile_pool(name="r1s", bufs=14) as r1spool,
            tc.tile_pool(name="rhs2", bufs=26) as rhs2pool,
            tc.tile_pool(name="sout", bufs=4) as soutpool,
            tc.tile_pool(name="ph2", bufs=3, space="PSUM") as ph2pool,
            tc.tile_pool(name="psc", bufs=2, space="PSUM") as pscpool,
        ):
            iw0 = 2 * JW if PP > 0 else JW
            cpk0_sb = cpool.tile([128, 32], f32)
            nc.sync.dma_start(out=cpk0_sb[:], in_=cpk[:, 0:32])
            hj0_sb = cpool.tile([128, iw0], bf16)
            nc.sync.dma_start(out=hj0_sb[:], in_=hj_pack[:, 0:iw0])
            cpk_sb = cpool.tile([128, NI * 32 + 1], f32)
            nc.sync.dma_start(out=cpk_sb[:], in_=cpk[:])
            cw_sb = cpool.tile([128, 32 + 256], bf16)
            nc.scalar.dma_start(out=cw_sb[:], in_=cw[:])
            w2p = cw_sb[:, 0:32]
            wm = cw_sb[:, 32 : 32 + 256]
            b2p = cpk_sb[:, NI * 32 : NI * 32 + 1]
            hjt = {0: hj0_sb}
            for it2 in range(1, NI):
                w2_ = (2 if it2 < PP else 1) * JW
                t_ = cpool.tile([128, w2_], bf16, name=f"hj{it2}")
                nc.sync.dma_start(
                    out=t_[:], in_=hj_pack[:, hj_off(it2) : hj_off(it2) + w2_]
                )
                hjt[it2] = t_

            k2 = 0
            ke = 0
            pending = []   # deferred mm2 spans of the previous item
            pend_fin = []  # deferred evac of the previous item
            pend_r2 = []   # deferred DVE relu2 (emitted one section late)
            for it in range(NI):
                is_pair = it < PP
                nsides = 2 if is_pair else 1
                iw = nsides * JW
                hj_it = hjt[it][:, 0:iw]
                rhs2 = {}
                for s in range(NS):
                    rhs1 = {}
                    for c in range(4):
                        p = 4 * s + c
                        pool = r1ppool if is_pair else r1spool
                        r1 = pool.tile([128, iw], bf16, name="r1p" if is_pair else "r1s")
                        cp_c = (
                            cpk0_sb[:, p : p + 1]
                            if it == 0
                            else cpk_sb[:, it * 32 + p : it * 32 + p + 1]
                        )
                        nc.vector.tensor_scalar(
                            r1[:], hj_it, cp_c, 0.0, add, vmax,
                        )
                        rhs1[c] = r1
                    for side in range(nsides):
                        while pend_r2:
                            pend_r2.pop(0)()
                        h2 = ph2pool.tile([128, 1024], f32, name="h2")
                        for r in range(2):
                            for c in range(4):
                                nc.tensor.matmul(
                                    h2[32 * c : 32 * c + 32, 512 * r : 512 * r + JW],
                                    lhsT=w2p[64 * r : 64 * r + 64, :],
                                    rhs=rhs1[c][
                                        64 * r : 64 * r + 64,
                                        side * JW : side * JW + JW,
                                    ],
                                    start=True, stop=True,
                                    tile_position=(64 * r, 32 * c),
                                )
                        r2t = rhs2pool.tile([128, 2 * JW], bf16, name="r2")
                        h2_rd = h2[:].rearrange("p (g j) -> p g j", g=2)[:, :, 0:JW]
                        r2_wr = r2t[:].rearrange("p (g j) -> p g j", g=2)
                        if (k2 % 8) in RELU2_ACT:
                            nc.scalar.activation(
                                r2_wr, h2_rd, Relu, bias=b2p, scale=1.0
                            )
                        else:
                            def mk_r2(r2_wr, h2_rd):
                                def run():
                                    nc.vector.tensor_scalar(
                                        r2_wr, h2_rd, b2p, 0.0, add, vmax
                                    )
                                return run
                            pend_r2.append(mk_r2(r2_wr, h2_rd))
                        k2 += 1
                        rhs2[(s, side)] = r2t
                    if pending:
                        pending.pop(0)()
                while pend_r2:
                    pend_r2.pop(0)()
                # flush any leftover mm2 spans + evac of previous item
                for f in pending:
                    f()
                pending = []
                for f in pend_fin:
                    f()
                pend_fin = []

                # defer this item's mm2 + evac into the next item's s-loop
                sc = pscpool.tile([128, JW], f32, name="sc")

                def mk_span(sc, rhs2, side, t, nsides):
                    def run():
                        for c2 in range(4):
                            g = 4 * t + c2
                            s_, r_ = g >> 1, g & 1
                            nc.tensor.matmul(
                                sc[32 * c2 : 32 * c2 + 32, :],
                                lhsT=wm[
                                    :, 128 * side + 32 * t : 128 * side + 32 * t + 32
                                ],
                                rhs=rhs2[(s_, side)][:, JW * r_ : JW * r_ + JW],
                                start=(t == 0 and side == 0),
                                stop=(t == NT - 1 and side == nsides - 1),
                                tile_position=(0, 32 * c2),
                                skip_group_check=True,
                            )
                    return run

                for side in range(nsides):
                    for t in range(NT):
                        pending.append(mk_span(sc, rhs2, side, t, nsides))

                def mk_fin(sc, it, ke):
                    def run():
                        so = soutpool.tile([128, JW], bf16, name="so")
                        if (ke % 8) in EVAC_ACT:
                            nc.scalar.activation(so[:], sc[:], Copy, bias=0.0, scale=1.0)
                        else:
                            nc.vector.tensor_scalar(so[:], sc[:], 0.0, None, add)
                        nc.sync.dma_start(
                            out=out[it * 128 : (it + 1) * 128, :], in_=so[:]
                        )
                    return run

                pend_fin.append(mk_fin(sc, it, ke))
                ke += 1
            for f in pending:
                f()
            for f in pend_fin:
                f()

    nc.finalize()
    return nc


def _get_nc(PP, SS):
    key = ("nc", PP, SS)
    if key not in _CACHE:
        _CACHE[key] = _build_bass(PP, SS)
    return _CACHE[key]


def _score_perm(side):
    """partition P -> i within a unit for the given side; -1 unused."""
    perm = -np.ones(128, np.int64)
    for t in range(IB // 16):
        for c2 in range(4):
            g = 4 * t + c2
            s_, r_ = g >> 1, g & 1
            for q in range(4):
                perm[32 * c2 + 16 * side + 4 * t + q] = 8 * s_ + 2 * q + r_
    return perm


def kernel(
    subj_idx, rel_idx, obj_idx, subj_table, rel_table, obj_table,
    proj_w, proj_b, w1, b1, w2, b2, w3, b3,
):
    from concourse.bass_utils import run_bass_kernel_spmd

    subj_idx = np.asarray(subj_idx)
    rel_idx = np.asarray(rel_idx)
    obj_idx = np.asarray(obj_idx)
    subj_table = np.asarray(subj_table, np.float32)
    rel_table = np.asarray(rel_table, np.float32)
    obj_table = np.asarray(obj_table, np.float32)
    proj_w = np.asarray(proj_w, np.float32)
    proj_b = np.asarray(proj_b, np.float32)
    w1 = np.asarray(w1, np.float32)
    b1 = np.asarray(b1, np.float32)
    w2 = np.asarray(w2, np.float32)
    b2 = np.asarray(b2, np.float32)
    w3 = np.asarray(w3, np.float32)
    b3 = np.asarray(b3, np.float32)

    NN = subj_idx.shape[0]

    key = (subj_idx.astype(np.int64) * rel_table.shape[0] + rel_idx) * obj_table.shape[
        0
    ] + obj_idx
    ukey, inv = np.unique(key, return_inverse=True)
    Uq = len(ukey)
    us = (ukey // (rel_table.shape[0] * obj_table.shape[0])).astype(np.int64)
    ur = ((ukey // obj_table.shape[0]) % rel_table.shape[0]).astype(np.int64)
    uo = (ukey % obj_table.shape[0]).astype(np.int64)

    pos = np.arange(NN)
    first = np.full(Uq, NN, np.int64)
    last = np.full(Uq, -1, np.int64)
    np.minimum.at(first, inv, pos)
    np.maximum.at(last, inv, pos)
    ro = np.argsort(first, kind="stable")
    co = np.argsort(last, kind="stable")
    F = first[ro]
    L = last[co]
    cut = np.searchsorted(L, F, side="right")

    combined = np.concatenate(
        [subj_table[us], rel_table[ur], obj_table[uo]], axis=-1
    )
    e = combined @ proj_w.T + proj_b
    w1a, w1b = w1[:, :D], w1[:, D:]
    hi = e @ w1a.T
    hj = e @ w1b.T

    n_ib = (Uq + IB - 1) // IB
    n_ju = (Uq + JW - 1) // JW
    ipad = n_ib * IB
    jpad = n_ju * JW

    # staircase -> pair/single items
    pairs = []
    singles = []
    for b in range(n_ib):
        u0 = int(cut[b * IB]) // JW if b * IB < Uq else 0
        w = n_ju - u0
        k = u0
        if w >= 2:
            pairs.append((b, u0))
            k = u0 + 2
        for u in range(k, n_ju):
            singles.append((b, u))
    PP = len(pairs) // N_CORES
    for (b, u0) in pairs[N_CORES * PP :]:
        singles.append((b, u0))
        singles.append((b, u0 + 1))
    pairs = pairs[: N_CORES * PP]
    SS = (len(singles) + N_CORES - 1) // N_CORES
    singles = singles + [singles[0]] * (N_CORES * SS - len(singles))
    NI = PP + SS

    C = np.zeros((ipad, D), np.float32)
    C[:Uq] = hi[ro] + b1
    hjT = np.zeros((D, jpad), np.float32)
    hjT[:, :Uq] = hj[co].T
    hj2 = np.concatenate([hjT, hjT], axis=0)  # [128, jpad]

    w2p = np.zeros((128, 32), np.float32)
    w2p[:64] = w2.T
    w2p[64:] = w2.T
    wm = np.zeros((128, 256), np.float32)
    for side in range(2):
        for t in range(4):
            for q in range(4):
                wm[
                    32 * q : 32 * q + 32, 128 * side + 32 * t + 16 * side + 4 * t + q
                ] = w3[0]
    cw = np.concatenate([w2p, wm], axis=1).astype(BF16)
    b2p = np.tile(b2, 4).reshape(128, 1).astype(np.float32)

    in_maps = []
    for cco in range(N_CORES):
        items = [("p", pairs[cco + N_CORES * k]) for k in range(PP)] + [
            ("s", singles[cco + N_CORES * k]) for k in range(SS)
        ]
        hjw = PP * 2 * JW + SS * JW
        hj_pack = np.zeros((128, hjw), np.float32)
        cpp = np.zeros((128, NI * 32 + 1), np.float32)
        o = 0
        for iidx, (kind, (b, u0)) in enumerate(items):
            w = 2 * JW if kind == "p" else JW
            hj_pack[:, o : o + w] = hj2[:, u0 * JW : u0 * JW + w]
            o += w
            for p in range(32):
                cpp[:64, iidx * 32 + p] = C[IB * b + 2 * p]
                cpp[64:, iidx * 32 + p] = C[IB * b + 2 * p + 1]
        cpp[:, NI * 32 : NI * 32 + 1] = b2p
        in_maps.append(
            {"hj_pack": hj_pack.astype(BF16), "cw": cw, "cpk": cpp}
        )

    nc = _get_nc(PP, SS)
    res = run_bass_kernel_spmd(
        nc, in_maps, core_ids=list(range(N_CORES)), **_CACHE.get("run_kwargs", {})
    )
    _CACHE["last_result"] = res

    sels = []
    for side in range(2):
        perm = _score_perm(side)
        sels.append(np.argsort(np.where(perm < 0, 10 ** 6, perm))[:IB])

    ugrid = np.zeros((ipad, jpad), np.float32)
    seen = set()
    for cco in range(N_CORES):
        items = [("p", pairs[cco + N_CORES * k]) for k in range(PP)] + [
            ("s", singles[cco + N_CORES * k]) for k in range(SS)
        ]
        out_c = res.results[cco]["out"].reshape(NI, 128, JW)
        for iidx, (kind, (b, u0)) in enumerate(items):
            nsides = 2 if kind == "p" else 1
            for side in range(nsides):
                u = u0 + side
                if (b, u) in seen:
                    continue
                seen.add((b, u))
                ugrid[b * IB : (b + 1) * IB, u * JW : (u + 1) * JW] = out_c[iidx][
                    sels[side]
                ].astype(np.float32)
    ugrid = 1.0 / (1.0 + np.exp(-(ugrid.astype(np.float64) + b3[0])))

    rowrank = np.empty(Uq, np.int64)
    rowrank[ro] = np.arange(Uq)
    colrank = np.empty(Uq, np.int64)
    colrank[co] = np.arange(Uq)
    scores = ugrid[rowrank[inv][:, None], colrank[inv][None, :]].astype(np.float32)
    return np.triu(scores, k=1)
